# revision 32
# baseline (speedup 1.0000x reference)
"""Multi-head attention (B=4, H=16, S=2048, D=128, causal+pad mask) on 8 TRN2 NeuronCores.

Sharding: the 64 (batch, head) pairs are split 8 per core (pure data parallel —
attention is independent per head, no collectives needed).

Per-core kernel (per head):
  - scores are computed TRANSPOSED: S^T[k, q] = K_block^T^T @ Q^T with the
    contraction dim d=128 on partitions, the k-block (128) as the PSUM partition
    dim and the allowed q-column range of the block (128-col granular, derived
    from the actual mask) as the moving dim. Q/K are host-cast to bf16.
  - Consecutive score segments are packed into [128, <=1024] PSUM group tiles
    (2 banks) so ONE scalar-engine ACTIVATE computes exp(scale*s) for the whole
    group straight out of PSUM into SBUF bf16 — amortizing the ~352-cycle ACT
    pipeline fill. No max-subtraction: scores*scale ~ N(0,1), exp is safe.
  - Partially-masked 128x128 chunks are zeroed by a bf16 multiply with
    host-derived deduped mask tiles on the vector engine. Fully-masked chunks
    are never computed; fully-allowed chunks are untouched.
  - P^T lands exactly in the layout the PV matmul needs (k on partitions):
    O[q_sub 128, 132] += P^T[:, chunk]^T @ V'[k_block] accumulated over k
    blocks in PSUM, where V' is V in bf16 with a ones column appended at col
    128 — so O[:, 128] is the softmax denominator for free.
  - reciprocal + per-partition scale normalizes, then DMA out as f32.
"""

import os
import sys
from collections import defaultdict

import numpy as np

try:  # the repo root that provides `concourse` / `gauge`
    import concourse.bass  # noqa: F401
except ImportError:  # pragma: no cover
    for _p in ("/opt/trn_rl_repo", "/root/.axon_site/_ro/trn_rl_repo"):
        if os.path.isdir(_p) and _p not in sys.path:
            sys.path.insert(0, _p)

import ml_dtypes

B, H, S, D = 4, 16, 2048, 128
BH = B * H
NCORES = 8
HPC = BH // NCORES  # heads per core = 8
QM = 256  # q megatile width; q sub-chunks of 128 map to PV output tiles
CH = 128  # q chunk granularity (PV stationary width / mask tile width)
KB = 128  # k block (PSUM partition dim of S^T)
NM = S // QM  # 8 q megatiles
NKB = S // KB  # 16 k blocks
VW = D + 4  # V' width: col D holds ones (softmax denom), cols D+1.. are zero pad
GCOLS = 1024  # exp group columns: [128, 1024] f32 = 2 PSUM banks
SCALE = float(np.float32(1.0 / np.sqrt(np.float32(D))))
NSUB = QM // CH  # q sub-chunks per megatile = 2

_CACHE: dict = {}
LAST_RESULTS = None  # BassKernelResults of the most recent run (for test harness)


def _derive_schedule(attn_mask):
    """Derive the packed block schedule from the actual mask.

    Returns (stream, contrib, mask_tiles):
      stream: ordered [(m, j, lo, hi, cmasks)] score segments; [lo, hi) is the
        128-granular allowed q-col range of block (m, j); cmasks lists
        (chunk_offset_within_seg, mask_id) for partially-masked 128-chunks.
      contrib: {(m, sub): n} count of PV contributions per output sub-tile.
      mask_tiles: [128, n_masks, CH] bf16 deduped transposed chunk masks.
    """
    am = np.asarray(attn_mask) != 0  # [S(q), S(k)]
    uniq: dict = {}
    tiles = []
    stream = []
    contrib: dict = defaultdict(int)
    for m in range(NM):
        for j in range(NKB):
            chunks = []
            for c in range(NSUB):
                cm = am[m * QM + c * CH : m * QM + (c + 1) * CH, j * KB : (j + 1) * KB]
                if not cm.any():
                    chunks.append(None)
                elif cm.all():
                    chunks.append("f")
                else:
                    key = cm.tobytes()
                    if key not in uniq:
                        uniq[key] = len(tiles)
                        tiles.append(cm.T.astype(ml_dtypes.bfloat16))  # [KB, CH]
                    chunks.append(uniq[key])
            c = 0
            while c < NSUB:
                if chunks[c] is None:
                    c += 1
                    continue
                c0 = c
                while c < NSUB and chunks[c] is not None:
                    c += 1
                cmasks = [
                    (cc - c0, chunks[cc]) for cc in range(c0, c) if chunks[cc] != "f"
                ]
                stream.append((m, j, c0 * CH, c * CH, cmasks))
                for cc in range(c0, c):
                    contrib[(m, cc)] += 1
    mask_tiles = np.stack(tiles, axis=1) if tiles else None  # [128, n, CH]
    return stream, dict(contrib), mask_tiles


def _build_program(stream, contrib, n_masks, use_pad):
    import concourse.mybir as mybir
    import concourse.tile as tile
    from concourse import bacc

    f32 = mybir.dt.float32
    bf16 = mybir.dt.bfloat16
    Exp = mybir.ActivationFunctionType.Exp

    nc = bacc.Bacc(None)
    qt_ext = nc.declare_dram_parameter("qt", [HPC, 128, S], bf16, isOutput=False)
    kt_ext = nc.declare_dram_parameter("kt", [HPC, 128, S], bf16, isOutput=False)
    vp_ext = nc.declare_dram_parameter("vp", [HPC, 128, NKB, VW], bf16, isOutput=False)
    if n_masks:
        mk_ext = nc.declare_dram_parameter("mk", [128, n_masks, CH], bf16, isOutput=False)
    if use_pad:
        pc_ext = nc.declare_dram_parameter("pc", [128, NKB], f32, isOutput=False)
    out_ext = nc.declare_dram_parameter("out", [HPC, S, D], f32, isOutput=True)

    # pack the segment stream into exp groups of <= GCOLS columns
    def pack(segs):
        gs = []
        cur, cols = [], 0
        for seg in segs:
            w = seg[3] - seg[2]
            if cols + w > GCOLS:
                gs.append((cur, cols))
                cur, cols = [], 0
            cur.append(seg)
            cols += w
        if cur:
            gs.append((cur, cols))
        return gs

    groups_fwd = pack(stream)

    with tile.TileContext(nc) as tc:
        with (
            tc.tile_pool(name="qt", bufs=3) as qt_pool,
            tc.tile_pool(name="kt", bufs=3) as kt_pool,
            tc.tile_pool(name="vp", bufs=3) as vp_pool,
            tc.tile_pool(name="pt", bufs=6) as pt_pool,
            tc.tile_pool(name="osb", bufs=6) as osb_pool,
            tc.tile_pool(name="rec", bufs=6) as rec_pool,
            tc.tile_pool(name="mk", bufs=1) as mk_pool,
            tc.tile_pool(name="st", bufs=2, space="PSUM") as st_pool,
            tc.tile_pool(name="ops", bufs=4, space="PSUM") as o_pool,
        ):
            # PE warm-up: ~3.8us of dummy matmuls during the DMA prologue trips
            # the HAM clock gate to 2.4 GHz before the first real matmul
            warm = mk_pool.tile([128, 512], bf16, name="warm")
            nc.gpsimd.memset(warm[:], 0.0)
            wo = st_pool.tile([128, GCOLS], f32, tag="st", name="wo")
            for wi in range(7):
                nc.tensor.matmul(
                    wo[:, 0:512], lhsT=warm[:, 0:128], rhs=warm[:], start=True, stop=True
                )

            if n_masks:
                mk = mk_pool.tile([128, n_masks, CH], bf16)
                nc.sync.dma_start(mk[:], mk_ext[:])
            if use_pad:
                pc = mk_pool.tile([128, NKB], f32)
                nc.sync.dma_start(pc[:], pc_ext[:])

            NQ = 4  # input DMA quarters — lets compute start before full tiles land
            for h in range(HPC):
                qt = qt_pool.tile([128, S], bf16)
                kt = kt_pool.tile([128, S], bf16)
                vp = vp_pool.tile([128, NKB, VW], bf16)
                qs = S // NQ
                js = NKB // NQ
                for q4 in range(NQ):
                    nc.sync.dma_start(
                        kt[:, q4 * qs : (q4 + 1) * qs], kt_ext[h, :, q4 * qs : (q4 + 1) * qs]
                    )
                    nc.sync.dma_start(
                        qt[:, q4 * qs : (q4 + 1) * qs], qt_ext[h, :, q4 * qs : (q4 + 1) * qs]
                    )
                    nc.sync.dma_start(
                        vp[:, q4 * js : (q4 + 1) * js, :],
                        vp_ext[h, :, q4 * js : (q4 + 1) * js, :],
                    )

                groups = groups_fwd
                o_tiles: dict = {}
                seen: dict = defaultdict(int)

                def finalize(m, sub, o):
                    rec = rec_pool.tile([128, 1], f32, name="rec")
                    nc.vector.reciprocal(rec[:], o[:, D : D + 1])
                    osb = osb_pool.tile([128, D], f32, name="osb")
                    nc.vector.tensor_scalar_mul(osb[:], o[:, 0:D], rec[:])
                    row0 = m * QM + sub * CH
                    nc.sync.dma_start(out_ext[h, row0 : row0 + CH, :], osb[:])

                def emit_scores(grp, gcols):
                    st = st_pool.tile([128, GCOLS], f32, tag="st", name="st")
                    p = 0
                    for m, j, lo, hi, cmasks in grp:
                        w = hi - lo
                        off = 0
                        while off < w:  # matmul output must not cross a PSUM bank
                            wseg = min(w - off, 512 - (p + off) % 512)
                            nc.tensor.matmul(
                                st[:, p + off : p + off + wseg],
                                lhsT=kt[:, j * KB : (j + 1) * KB],
                                rhs=qt[:, m * QM + lo + off : m * QM + lo + off + wseg],
                                start=True,
                                stop=True,
                            )
                            off += wseg
                        p += w
                    pt = pt_pool.tile([128, GCOLS], bf16, tag="pt", name="pt")
                    nc.scalar.activation(pt[:, :gcols], st[:, :gcols], Exp, scale=SCALE)
                    return pt

                def emit_pv(grp, pt):
                    p = 0
                    for m, j, lo, hi, cmasks in grp:
                        w = hi - lo
                        for coff, mi in cmasks:
                            nc.vector.tensor_mul(
                                pt[:, p + coff * CH : p + (coff + 1) * CH],
                                pt[:, p + coff * CH : p + (coff + 1) * CH],
                                mk[:, mi, :],
                            )
                        if use_pad:
                            nc.vector.tensor_scalar_mul(
                                pt[:, p : p + w], pt[:, p : p + w], pc[:, j : j + 1]
                            )
                        if m not in o_tiles:
                            o_tiles[m] = [
                                o_pool.tile([128, VW], f32, tag="o", name=f"o{s_}")
                                for s_ in range(NSUB)
                            ]
                        for c in range(w // CH):
                            sub = lo // CH + c
                            key = (m, sub)
                            seen[key] += 1
                            nc.tensor.matmul(
                                o_tiles[m][sub][:],
                                lhsT=pt[:, p + c * CH : p + (c + 1) * CH],
                                rhs=vp[:, j, :],
                                start=seen[key] == 1,
                                stop=seen[key] == contrib[key],
                            )
                            if seen[key] == contrib[key]:
                                finalize(m, sub, o_tiles[m][sub])
                        p += w

                for grp, gcols in groups:
                    pt = emit_scores(grp, gcols)
                    emit_pv(grp, pt)
    nc.compile()
    return nc


def _prep_inputs(q, k, v, attn_mask, pad_mask):
    q = np.asarray(q, dtype=np.float32).reshape(BH, S, D)
    k = np.asarray(k, dtype=np.float32).reshape(BH, S, D)
    v = np.asarray(v, dtype=np.float32).reshape(BH, S, D)

    qt = np.ascontiguousarray(q.transpose(0, 2, 1)).astype(ml_dtypes.bfloat16)
    kt = np.ascontiguousarray(k.transpose(0, 2, 1)).astype(ml_dtypes.bfloat16)

    # V': [BH, 128(row within k block), NKB, VW] bf16; col D = 1.0 (denominator)
    vp = np.zeros((BH, 128, NKB, VW), dtype=ml_dtypes.bfloat16)
    vblocks = v.reshape(BH, NKB, 128, D).transpose(0, 2, 1, 3)
    vp[:, :, :, :D] = vblocks.astype(ml_dtypes.bfloat16)
    vp[:, :, :, D] = 1.0

    pad = np.asarray(pad_mask).reshape(B, S) != 0
    use_pad = not bool(pad.all())
    pcs = None
    if use_pad:
        pcs = []
        for c in range(NCORES):
            b = (c * HPC) // H
            pcs.append(
                np.ascontiguousarray(pad[b].reshape(NKB, 128).T.astype(np.float32))
            )
    return qt, kt, vp, use_pad, pcs


def kernel(q, k, v, attn_mask, pad_mask):
    global LAST_RESULTS
    from concourse.bass_utils import run_bass_kernel_spmd

    try:  # tracing needs the NTFF hook; without it BASS_TRACE=1 would crash
        import antenv.axon_hooks  # noqa: F401
    except ImportError:
        os.environ["BASS_NEVER_TRACE"] = "1"

    stream, contrib, mask_tiles = _derive_schedule(attn_mask)
    qt, kt, vp, use_pad, pcs = _prep_inputs(q, k, v, attn_mask, pad_mask)
    n_masks = 0 if mask_tiles is None else mask_tiles.shape[1]

    key = (np.asarray(attn_mask).tobytes(), use_pad)
    nc = _CACHE.get(key)
    if nc is None:
        nc = _build_program(stream, contrib, n_masks, use_pad)
        _CACHE[key] = nc

    in_maps = []
    for c in range(NCORES):
        sl = slice(c * HPC, (c + 1) * HPC)
        m = {"qt": qt[sl], "kt": kt[sl], "vp": vp[sl]}
        if n_masks:
            m["mk"] = mask_tiles
        if use_pad:
            m["pc"] = pcs[c]
        in_maps.append(m)

    res = run_bass_kernel_spmd(nc, in_maps, core_ids=list(range(NCORES)))
    LAST_RESULTS = res
    out = np.concatenate([res.results[c]["out"] for c in range(NCORES)], axis=0)
    return np.ascontiguousarray(out.reshape(B, H, S, D).astype(np.float32))


# revision 33
# speedup vs baseline: 1.0323x; 1.0323x over previous
"""Multi-head attention (B=4, H=16, S=2048, D=128, causal+pad mask) on 8 TRN2 NeuronCores.

Sharding: the 64 (batch, head) pairs are split 8 per core (pure data parallel —
attention is independent per head, no collectives needed).

Per-core kernel (per head):
  - scores are computed TRANSPOSED: S^T[k, q] = K_block^T^T @ Q^T with the
    contraction dim d=128 on partitions, the k-block (128) as the PSUM partition
    dim and the allowed q-column range of the block (128-col granular, derived
    from the actual mask) as the moving dim. Q/K are host-cast to bf16.
  - Consecutive score segments are packed into [128, <=1024] PSUM group tiles
    (2 banks) so ONE scalar-engine ACTIVATE computes exp(scale*s) for the whole
    group straight out of PSUM into SBUF bf16 — amortizing the ~352-cycle ACT
    pipeline fill. No max-subtraction: scores*scale ~ N(0,1), exp is safe.
  - Partially-masked 128x128 chunks are zeroed by a bf16 multiply with
    host-derived deduped mask tiles on the vector engine. Fully-masked chunks
    are never computed; fully-allowed chunks are untouched.
  - P^T lands exactly in the layout the PV matmul needs (k on partitions):
    O[q_sub 128, 132] += P^T[:, chunk]^T @ V'[k_block] accumulated over k
    blocks in PSUM, where V' is V in bf16 with a ones column appended at col
    128 — so O[:, 128] is the softmax denominator for free.
  - reciprocal + per-partition scale normalizes, then DMA out as f32.
"""

import os
import sys
from collections import defaultdict

import numpy as np

try:  # the repo root that provides `concourse` / `gauge`
    import concourse.bass  # noqa: F401
except ImportError:  # pragma: no cover
    for _p in ("/opt/trn_rl_repo", "/root/.axon_site/_ro/trn_rl_repo"):
        if os.path.isdir(_p) and _p not in sys.path:
            sys.path.insert(0, _p)

import ml_dtypes

B, H, S, D = 4, 16, 2048, 128
BH = B * H
NCORES = 8
HPC = BH // NCORES  # heads per core = 8
QM = 256  # q megatile width; q sub-chunks of 128 map to PV output tiles
CH = 128  # q chunk granularity (PV stationary width / mask tile width)
KB = 128  # k block (PSUM partition dim of S^T)
NM = S // QM  # 8 q megatiles
NKB = S // KB  # 16 k blocks
VW = D + 4  # V' width: col D holds ones (softmax denom), cols D+1.. are zero pad
GCOLS = 1024  # exp group columns: [128, 1024] f32 = 2 PSUM banks
SCALE = float(np.float32(1.0 / np.sqrt(np.float32(D))))
NSUB = QM // CH  # q sub-chunks per megatile = 2

_CACHE: dict = {}
LAST_RESULTS = None  # BassKernelResults of the most recent run (for test harness)


def _derive_schedule(attn_mask):
    """Derive the packed block schedule from the actual mask.

    Returns (stream, contrib, mask_tiles):
      stream: ordered [(m, j, lo, hi, cmasks)] score segments; [lo, hi) is the
        128-granular allowed q-col range of block (m, j); cmasks lists
        (chunk_offset_within_seg, mask_id) for partially-masked 128-chunks.
      contrib: {(m, sub): n} count of PV contributions per output sub-tile.
      mask_tiles: [128, n_masks, CH] bf16 deduped transposed chunk masks.
    """
    am = np.asarray(attn_mask) != 0  # [S(q), S(k)]
    uniq: dict = {}
    tiles = []
    stream = []
    contrib: dict = defaultdict(int)
    for m in range(NM):
        for j in range(NKB):
            chunks = []
            for c in range(NSUB):
                cm = am[m * QM + c * CH : m * QM + (c + 1) * CH, j * KB : (j + 1) * KB]
                if not cm.any():
                    chunks.append(None)
                elif cm.all():
                    chunks.append("f")
                else:
                    key = cm.tobytes()
                    if key not in uniq:
                        uniq[key] = len(tiles)
                        tiles.append(cm.T.astype(ml_dtypes.bfloat16))  # [KB, CH]
                    chunks.append(uniq[key])
            c = 0
            while c < NSUB:
                if chunks[c] is None:
                    c += 1
                    continue
                c0 = c
                while c < NSUB and chunks[c] is not None:
                    c += 1
                cmasks = [
                    (cc - c0, chunks[cc]) for cc in range(c0, c) if chunks[cc] != "f"
                ]
                stream.append((m, j, c0 * CH, c * CH, cmasks))
                for cc in range(c0, c):
                    contrib[(m, cc)] += 1
    mask_tiles = np.stack(tiles, axis=1) if tiles else None  # [128, n, CH]
    return stream, dict(contrib), mask_tiles


def _build_program(stream, contrib, n_masks, use_pad):
    import concourse.mybir as mybir
    import concourse.tile as tile
    from concourse import bacc

    f32 = mybir.dt.float32
    bf16 = mybir.dt.bfloat16
    Exp = mybir.ActivationFunctionType.Exp

    nc = bacc.Bacc(None)
    qt_ext = nc.declare_dram_parameter("qt", [HPC, 128, S], bf16, isOutput=False)
    kt_ext = nc.declare_dram_parameter("kt", [HPC, 128, S], bf16, isOutput=False)
    vp_ext = nc.declare_dram_parameter("vp", [HPC, 128, NKB, VW], bf16, isOutput=False)
    if n_masks:
        mk_ext = nc.declare_dram_parameter("mk", [128, n_masks, CH], bf16, isOutput=False)
    if use_pad:
        pc_ext = nc.declare_dram_parameter("pc", [128, NKB], f32, isOutput=False)
    out_ext = nc.declare_dram_parameter("out", [HPC, S, D], f32, isOutput=True)

    # pack the segment stream into exp groups of <= GCOLS columns
    def pack(segs):
        gs = []
        cur, cols = [], 0
        for seg in segs:
            w = seg[3] - seg[2]
            if cols + w > GCOLS:
                gs.append((cur, cols))
                cur, cols = [], 0
            cur.append(seg)
            cols += w
        if cur:
            gs.append((cur, cols))
        return gs

    groups_fwd = pack(stream)

    with tile.TileContext(nc) as tc:
        with (
            tc.tile_pool(name="qt", bufs=2) as qt_pool,
            tc.tile_pool(name="kt", bufs=2) as kt_pool,
            tc.tile_pool(name="vp", bufs=2) as vp_pool,
            tc.tile_pool(name="pt", bufs=6) as pt_pool,
            tc.tile_pool(name="osb", bufs=6) as osb_pool,
            tc.tile_pool(name="rec", bufs=6) as rec_pool,
            tc.tile_pool(name="mk", bufs=1) as mk_pool,
            tc.tile_pool(name="st", bufs=2, space="PSUM") as st_pool,
            tc.tile_pool(name="ops", bufs=4, space="PSUM") as o_pool,
        ):
            # PE warm-up: ~3.8us of dummy matmuls during the DMA prologue trips
            # the HAM clock gate to 2.4 GHz before the first real matmul
            warm = mk_pool.tile([128, 512], bf16, name="warm")
            nc.gpsimd.memset(warm[:], 0.0)
            wo = st_pool.tile([128, GCOLS], f32, tag="st", name="wo")
            for wi in range(7):
                nc.tensor.matmul(
                    wo[:, 0:512], lhsT=warm[:, 0:128], rhs=warm[:], start=True, stop=True
                )

            if n_masks:
                mk = mk_pool.tile([128, n_masks, CH], bf16)
                nc.sync.dma_start(mk[:], mk_ext[:])
            if use_pad:
                pc = mk_pool.tile([128, NKB], f32)
                nc.sync.dma_start(pc[:], pc_ext[:])

            NQ = 4  # input DMA quarters — lets compute start before full tiles land
            for h in range(HPC):
                qt = qt_pool.tile([128, S], bf16)
                kt = kt_pool.tile([128, S], bf16)
                vp = vp_pool.tile([128, NKB, VW], bf16)
                qs = S // NQ
                js = NKB // NQ
                for q4 in range(NQ):
                    nc.sync.dma_start(
                        kt[:, q4 * qs : (q4 + 1) * qs], kt_ext[h, :, q4 * qs : (q4 + 1) * qs]
                    )
                    nc.sync.dma_start(
                        qt[:, q4 * qs : (q4 + 1) * qs], qt_ext[h, :, q4 * qs : (q4 + 1) * qs]
                    )
                    nc.sync.dma_start(
                        vp[:, q4 * js : (q4 + 1) * js, :],
                        vp_ext[h, :, q4 * js : (q4 + 1) * js, :],
                    )

                groups = groups_fwd
                o_tiles: dict = {}
                seen: dict = defaultdict(int)

                def finalize(m, sub, o):
                    rec = rec_pool.tile([128, 1], f32, name="rec")
                    nc.vector.reciprocal(rec[:], o[:, D : D + 1])
                    osb = osb_pool.tile([128, D], f32, name="osb")
                    nc.vector.tensor_scalar_mul(osb[:], o[:, 0:D], rec[:])
                    row0 = m * QM + sub * CH
                    nc.sync.dma_start(out_ext[h, row0 : row0 + CH, :], osb[:])

                def emit_scores(grp, gcols):
                    st = st_pool.tile([128, GCOLS], f32, tag="st", name="st")
                    p = 0
                    for m, j, lo, hi, cmasks in grp:
                        w = hi - lo
                        off = 0
                        while off < w:  # matmul output must not cross a PSUM bank
                            wseg = min(w - off, 512 - (p + off) % 512)
                            nc.tensor.matmul(
                                st[:, p + off : p + off + wseg],
                                lhsT=kt[:, j * KB : (j + 1) * KB],
                                rhs=qt[:, m * QM + lo + off : m * QM + lo + off + wseg],
                                start=True,
                                stop=True,
                            )
                            off += wseg
                        p += w
                    pt = pt_pool.tile([128, GCOLS], bf16, tag="pt", name="pt")
                    nc.scalar.activation(pt[:, :gcols], st[:, :gcols], Exp, scale=SCALE)
                    return pt

                def emit_pv(grp, pt):
                    p = 0
                    for m, j, lo, hi, cmasks in grp:
                        w = hi - lo
                        for coff, mi in cmasks:
                            nc.vector.tensor_mul(
                                pt[:, p + coff * CH : p + (coff + 1) * CH],
                                pt[:, p + coff * CH : p + (coff + 1) * CH],
                                mk[:, mi, :],
                            )
                        if use_pad:
                            nc.vector.tensor_scalar_mul(
                                pt[:, p : p + w], pt[:, p : p + w], pc[:, j : j + 1]
                            )
                        if m not in o_tiles:
                            o_tiles[m] = [
                                o_pool.tile([128, VW], f32, tag="o", name=f"o{s_}")
                                for s_ in range(NSUB)
                            ]
                        for c in range(w // CH):
                            sub = lo // CH + c
                            key = (m, sub)
                            seen[key] += 1
                            nc.tensor.matmul(
                                o_tiles[m][sub][:],
                                lhsT=pt[:, p + c * CH : p + (c + 1) * CH],
                                rhs=vp[:, j, :],
                                start=seen[key] == 1,
                                stop=seen[key] == contrib[key],
                            )
                            if seen[key] == contrib[key]:
                                finalize(m, sub, o_tiles[m][sub])
                        p += w

                for grp, gcols in groups:
                    pt = emit_scores(grp, gcols)
                    emit_pv(grp, pt)
    nc.compile()
    return nc


def _prep_inputs(q, k, v, attn_mask, pad_mask):
    q = np.asarray(q, dtype=np.float32).reshape(BH, S, D)
    k = np.asarray(k, dtype=np.float32).reshape(BH, S, D)
    v = np.asarray(v, dtype=np.float32).reshape(BH, S, D)

    qt = np.ascontiguousarray(q.transpose(0, 2, 1)).astype(ml_dtypes.bfloat16)
    kt = np.ascontiguousarray(k.transpose(0, 2, 1)).astype(ml_dtypes.bfloat16)

    # V': [BH, 128(row within k block), NKB, VW] bf16; col D = 1.0 (denominator)
    vp = np.zeros((BH, 128, NKB, VW), dtype=ml_dtypes.bfloat16)
    vblocks = v.reshape(BH, NKB, 128, D).transpose(0, 2, 1, 3)
    vp[:, :, :, :D] = vblocks.astype(ml_dtypes.bfloat16)
    vp[:, :, :, D] = 1.0

    pad = np.asarray(pad_mask).reshape(B, S) != 0
    use_pad = not bool(pad.all())
    pcs = None
    if use_pad:
        pcs = []
        for c in range(NCORES):
            b = (c * HPC) // H
            pcs.append(
                np.ascontiguousarray(pad[b].reshape(NKB, 128).T.astype(np.float32))
            )
    return qt, kt, vp, use_pad, pcs


def kernel(q, k, v, attn_mask, pad_mask):
    global LAST_RESULTS
    from concourse.bass_utils import run_bass_kernel_spmd

    try:  # tracing needs the NTFF hook; without it BASS_TRACE=1 would crash
        import antenv.axon_hooks  # noqa: F401
    except ImportError:
        os.environ["BASS_NEVER_TRACE"] = "1"

    stream, contrib, mask_tiles = _derive_schedule(attn_mask)
    qt, kt, vp, use_pad, pcs = _prep_inputs(q, k, v, attn_mask, pad_mask)
    n_masks = 0 if mask_tiles is None else mask_tiles.shape[1]

    key = (np.asarray(attn_mask).tobytes(), use_pad)
    nc = _CACHE.get(key)
    if nc is None:
        nc = _build_program(stream, contrib, n_masks, use_pad)
        _CACHE[key] = nc

    in_maps = []
    for c in range(NCORES):
        sl = slice(c * HPC, (c + 1) * HPC)
        m = {"qt": qt[sl], "kt": kt[sl], "vp": vp[sl]}
        if n_masks:
            m["mk"] = mask_tiles
        if use_pad:
            m["pc"] = pcs[c]
        in_maps.append(m)

    res = run_bass_kernel_spmd(nc, in_maps, core_ids=list(range(NCORES)))
    LAST_RESULTS = res
    out = np.concatenate([res.results[c]["out"] for c in range(NCORES)], axis=0)
    return np.ascontiguousarray(out.reshape(B, H, S, D).astype(np.float32))
